# revision 33
# baseline (speedup 1.0000x reference)
"""TRN2 Bass kernel for nn_ClusteringLayer (vq_codebook).

Computes, for inputs x (131072, 256) and clusters c (256, 256):
    dist2[r,k] = ||x_r||^2 + ||c_k||^2 - 2 x_r.c_k
    q = 1/(1+dist2);  q = q / sum_k q          (ALPHA=1 -> power is a no-op)

Strategy (data-parallel over 8 NeuronCores, 16384 rows each):
  - PE does ONLY the -2 x.c product: 2 fp16 matmuls per 128-row block
    (contraction split over d in 2 chunks of 128). No aug matmul.
  - A custom fused DVE op (BIAS_RECIP_SUM_ANT) reads the PSUM product and
    in ONE pass adds c2+1 (second fp32 stream, a constant [128,256] tile),
    adds x2 per row (per-partition scalar AP), computes ~1/x via the
    bitwise-NOT exponent-flip seed + 1 Newton step (~1.7e-3 rel), and
    emits the row-sum s via the accumulate path. One DVE op per block
    replaces: aug matmul + reciprocal + reduce.
  - W3 (out = qun / s, fp16 out): ACT Copy scale=1/s on blocks 0,1 of each
    half (1/s via a small DVE reciprocal), GPSIMD normalize_recip on 2,3.
  - Host prep: x -> fp16 transposed to [d, r] with a per-supertile row
    permutation row = h*512 + p*4 + b so each half-supertile's fp16 output
    is DMA'd as 128 x 2KB contiguous DRAM lines; x2 in fp32 exact; fp16
    DRAM out is upcast to fp32 on host.
  - Matmult instructions can carry only ONE sync-wait: PE consts live in
    one DMA'd tile fenced by one dummy matmul; each supertile's xt DMA is
    fenced the same way.
"""

import os
import sys

for _p in ("/root/.axon_site/_ro/trn_rl_repo", "/opt/trn_rl_repo"):
    if os.path.isdir(_p) and _p not in sys.path:
        sys.path.append(_p)

import numpy as np

from concourse import bacc, tile
import concourse.mybir as mybir
from concourse.bass_utils import run_bass_kernel_spmd

F32 = mybir.dt.float32
F16 = mybir.dt.float16

# ---------------------------------------------------------------------------
# Custom fused DVE op:
#   x   = in0 + in1 + s0          (psum product + (c2+1) stream + x2 scalar)
#   nx  = bitcast(~x)             (exponent-flip reciprocal seed)
#   y0  = nx * s1;  out = y0 * (imm2 - x * y0)   (one Newton step, ~1.7e-3)
#   accum_out = sum(out) per partition
# Registered into dve_ops at import (the documented extension point is
# appending to OPS; done here since kernel.py must be self-contained).
# ---------------------------------------------------------------------------
import concourse.dve_ops as dve_ops
from concourse.dve_ops import DveOp
from concourse.dve_spec import (
    Spec, Src0, Src1, C0, C1, C2, Zero, AluOp, Bin, lower, _has_src1,
)
from concourse.dve_uop import DveOpSpec
from operator import add as _add

RECIP_C0 = -0.23549792   # Chebyshev seed scale for t = x*bitcast(~x) in [-4.5,-4]
RECIP_C1 = 2.0017324     # Newton-step constant


def _bias_recip_sum_ref(in0, in1, s0, s1, imm2):
    x = (in0.astype(np.float32) + in1 + np.float32(s0)).astype(np.float32)
    nx = (~x.view(np.int32)).view(np.float32)
    y0 = (nx * np.float32(s1)).astype(np.float32)
    b = (y0 * (np.float32(imm2) - x * y0)).astype(np.float32)
    return b, b.reshape(b.shape[0], -1).sum(axis=-1, keepdims=True)


def _register_op():
    name = "BIAS_RECIP_SUM_ANT"
    if name in dve_ops._SUB_OPCODE_FOR_NAME:
        return next(op for op in dve_ops.OPS if op.name == name)
    _x = (Src0 + Src1) + C0
    _nx = Bin(AluOp.BITWISE_NOT, _x, _x)
    _y0 = _nx * C1
    spec = Spec(body=_y0 * (C2 - _x * _y0),
                accum=_add, accum_init=Zero, reference=_bias_recip_sum_ref)
    row = dve_ops._CUSTOM_DVE_ROW_BASE + len(dve_ops.OPS)
    assert row < 0x20
    shas = {}
    for ver in ("v3", "v4"):
        u = lower(spec, ver=ver)
        shas[ver] = DveOpSpec(name=name, opcode=row, uops=u,
                              rd1_en=_has_src1(spec)).sha(ver)
    op = DveOp(name, spec, subdim=False, uops_sha=shas)
    dve_ops.OPS.append(op)
    dve_ops.CUSTOM_DVE_SPECS[name] = spec
    dve_ops._SUB_OPCODE_FOR_NAME[name] = row
    return op


BIAS_RECIP_SUM_ANT = _register_op()

NCORES = 8
B = 131072
D = 256
K = 256
R = B // NCORES          # rows per core
S = 1024                 # rows per supertile
NB = S // 128            # 128-row blocks per supertile
NST = R // S             # supertiles per core
NCOL = R // 128          # x2p columns (one per block)
GSZ = 512                # warmup matmul free-dim size
KW = 512                 # konst tile: ct[p, ch*256+k] = -2*fp16(c)[k, ch*128+p]
WARMUP_MMS = 16

_nc_cache = None


def _build():
    nc = bacc.Bacc("TRN2", target_bir_lowering=False, debug=False,
                   num_devices=NCORES)
    xt_d = nc.dram_tensor("xt", [128, 2, R], F16, kind="ExternalInput").ap()
    x2p_d = nc.dram_tensor("x2p", [128, NCOL], F32, kind="ExternalInput").ap()
    c2b_d = nc.dram_tensor("c2b", [128, K], F32, kind="ExternalInput").ap()
    ko_d = nc.dram_tensor("ko", [128, KW], F16, kind="ExternalInput").ap()
    out_d = nc.dram_tensor("out", [R, K], F16, kind="ExternalOutput").ap()

    with tile.TileContext(nc) as tc:
        with (
            tc.tile_pool(name="const", bufs=1) as cpool,
            tc.tile_pool(name="xtp", bufs=NST) as xtpool,
            tc.tile_pool(name="qunp", bufs=8) as qunpool,
            tc.tile_pool(name="outp", bufs=8) as outpool,
            tc.tile_pool(name="sp", bufs=16) as spool,
            tc.tile_pool(name="rsp", bufs=8) as rspool,
            tc.tile_pool(name="qps", bufs=6, space="PSUM") as qpool,
            tc.tile_pool(name="x2ps", bufs=1, space="PSUM") as x2pool,
        ):
            ko_t = cpool.tile([128, KW], F16, tag="ko")
            nc.sync.dma_start(ko_t[:], ko_d[:])
            # first supertile's input right behind the PE consts so the
            # pipeline starts as early as possible
            xt_t0 = xtpool.tile([128, 2, S], F16, tag="xt")
            nc.sync.dma_start(xt_t0[:], xt_d[:, :, 0:S])
            xt_tiles = [xt_t0]
            c2b_t = cpool.tile([128, K], F32, tag="c2b")
            nc.sync.dma_start(c2b_t[:], c2b_d[:])
            x2c_t = cpool.tile([128, NCOL], F32, tag="x2c")
            nc.sync.dma_start(x2c_t[:], x2p_d[:])

            ct = ko_t[:, 0:512].rearrange("p (c k) -> p c k", c=2)

            # Prologue: one fence matmul absorbs the konst DMA wait. (No
            # warmup burst: the PE is far from critical, HAM ramps during
            # the first supertiles.)
            fence_p = x2pool.tile([1, GSZ], F32, tag="x2p")
            nc.tensor.matmul(fence_p[0:1, 0:8], ko_t[:, 0:1], ko_t[:, 0:8],
                             start=True, stop=True)

            # prefetch ALL xt supertiles up front so the in-order Sync queue
            # never delays an input DMA behind output DMAs
            for st in range(1, NST):
                xt_t = xtpool.tile([128, 2, S], F16, tag="xt")
                nc.sync.dma_start(xt_t[:], xt_d[:, :, st * S:(st + 1) * S])
                xt_tiles.append(xt_t)

            for st in range(NST):
                r0 = st * S
                xt_t = xt_tiles[st]

                # per-supertile fence absorbs the xt DMA wait
                nc.tensor.matmul(fence_p[0:1, 0:8], xt_t[:, 0, 0:1],
                                 xt_t[:, 0, 0:8], start=True, stop=True)

                for h in range(2):
                    qun_t = qunpool.tile([128, 4, K], F32, tag="qun")
                    s_g = spool.tile([128, 2], F32, tag="sg")
                    s_a = spool.tile([128, 2], F32, tag="sa")
                    rs_t = rspool.tile([128, 2], F32, tag="rs")
                    out_t = outpool.tile([128, 4, K], F16, tag="out")
                    # two 1-bank PSUM tiles per half: finer PE->DVE handoff
                    for t2 in range(2):
                        qp = qpool.tile([128, 2, K], F32, tag="qp")
                        for j in range(2):
                            b = 4 * h + 2 * t2 + j
                            nc.tensor.matmul(
                                qp[:, j, :],
                                xt_t[:, 0, b * 128:(b + 1) * 128],
                                ct[:, 0, :], start=True, stop=False,
                            )
                            nc.tensor.matmul(
                                qp[:, j, :],
                                xt_t[:, 1, b * 128:(b + 1) * 128],
                                ct[:, 1, :], start=False, stop=True,
                            )
                        # fused (+c2+1, +x2, recip, row-sum): 1 DVE op/block.
                        # Row sums go to per-engine-group tiles (s_g for the
                        # GPSIMD-normalized blocks, s_a for the ACT ones) so
                        # GPSIMD's 1/s write-back never false-shares with
                        # DVE accumulate writes or the DVE 1/s read.
                        for j in range(2):
                            jj = 2 * t2 + j
                            b = 4 * h + jj
                            acc = s_g if t2 == 0 else s_a
                            nc.vector._custom_dve(
                                BIAS_RECIP_SUM_ANT,
                                out=qun_t[:, jj, :], in0=qp[:, j, :],
                                in1=c2b_t[:],
                                s0=x2c_t[:, st * NB + b:st * NB + b + 1],
                                s1=RECIP_C0, imm2=RECIP_C1,
                                accum_out=acc[:, j:j + 1],
                            )
                        # W3: GPSIMD normalize_recip on blocks 0,1 of the
                        # half (start right as their fused ops land); ACT
                        # (Copy * 1/s) on blocks 2,3 (rs in one DVE op)
                        if t2 == 0:
                            for j in range(2):
                                nc.gpsimd.normalize_recip(
                                    out_t[:, j, :], qun_t[:, j, :],
                                    s_g[:, j:j + 1])
                        else:
                            nc.vector.reciprocal_approx_fast(
                                out=rs_t[:], in_=s_a[:])
                            for j in range(2):
                                nc.scalar.activation(
                                    out_t[:, 2 + j, :], qun_t[:, 2 + j, :],
                                    mybir.ActivationFunctionType.Copy,
                                    scale=rs_t[:, j:j + 1],
                                )

                    half = S // 2
                    nc.sync.dma_start(
                        out_d[r0 + h * half:r0 + (h + 1) * half, :]
                        .rearrange("(p b) k -> p b k", p=128),
                        out_t[:],
                    )
    nc.compile()
    return nc


def _get_nc():
    global _nc_cache
    if _nc_cache is None:
        _nc_cache = _build()
    return _nc_cache


def _prep_in_maps(inputs, clusters):
    x = np.asarray(inputs, dtype=np.float32)
    c = np.asarray(clusters, dtype=np.float32)

    xh = x.astype(np.float16)
    # Row permutation: global row st*1024 + h*512 + j*4 + b lives at device
    # position [dp, ch, st*1024 + (h*4+b)*128 + j]; partition j of block
    # (h, b) then holds row h*512 + j*4 + b, so each half-supertile's output
    # is contiguous 2KB per partition in DRAM row-major order.
    # xh rows decompose as (st, h, j, b) with strides (1024, 512, 4, 1).
    xt_all = np.ascontiguousarray(
        xh.reshape(NCORES, NST, 2, 128, 4, 2, 128)
        .transpose(0, 6, 5, 1, 2, 4, 3)
        .reshape(NCORES, 128, 2, R))
    # x2 (consistent with the fp16-rounded x) in fp32, laid out
    # [partition j, block col (st, h, b)].
    x2 = (xh.astype(np.float64) ** 2).sum(1).astype(np.float32)
    x2p_all = np.ascontiguousarray(
        x2.reshape(NCORES, NST, 2, 128, 4)
        .transpose(0, 3, 1, 2, 4)
        .reshape(NCORES, 128, NCOL))

    ch = c.astype(np.float16)
    c2b = np.broadcast_to(
        ((ch.astype(np.float64) ** 2).sum(1) + 1.0).astype(np.float32),
        (128, K)).copy()

    ko = np.ascontiguousarray(
        (-2.0 * ch.astype(np.float32)).astype(np.float16).T
    ).reshape(2, 128, K).transpose(1, 0, 2).reshape(128, 512)

    return [
        {"xt": xt_all[i], "x2p": x2p_all[i], "c2b": c2b, "ko": ko}
        for i in range(NCORES)
    ]


def _run(inputs, clusters, trace=False, tmpdir=None):
    nc = _get_nc()
    in_maps = _prep_in_maps(inputs, clusters)
    res = run_bass_kernel_spmd(nc, in_maps, list(range(NCORES)),
                               trace=trace, tmpdir=tmpdir)
    out = np.concatenate(
        [res.results[i]["out"] for i in range(NCORES)], axis=0
    ).astype(np.float32)
    return out, res


def kernel(inputs, clusters):
    out, _ = _run(inputs, clusters, trace=False)
    return out
